# revision 29
# baseline (speedup 1.0000x reference)
"""nn_EncoderModel: 2-layer LSTM encoder (B=128, T=512, E=256, H=1024)
on 8 trn2 NeuronCores — v2.4.

Hidden-dim model parallelism (core k owns h-dims [128k,128(k+1)) of
both layers), layer 1 lagged one step behind layer 0, ONE merged
AllGather per step carrying [h0(t); h1(t-1)] in bf16 (the AG here is
latency+size bound: 32KB ~6us vs 128KB ~14us serial). No per-step
length masking: states evolve freely past each sequence's end and the
output is captured at its firing step (outacc += msel_t * h1), which
is mathematically identical to dynamic_rnn's freeze-and-read-last. Matmuls run in float32r (1 cycle/row; fp32
is 4) or bf16. Cell elementwise math is fused into
scalar_tensor_tensor/tensor_scalar ops with the dynamic_rnn length mask
applied as per-partition scalars. Embedding lookup folds into the
layer-0 matmul via the one-hot trick; the one-hot is built directly in
[V, B] layout from a pre-transposed token table (no PE transpose).

Queues: PE = matmuls + state transposes; ACT = activations (+ ohT
fallback copy); DVE = cell elementwise + stage copies; gpsimd = cin
DMAs + collectives; SP = readbacks only. This keeps every compute
queue free of waits on collective completion.
"""

from contextlib import ExitStack

import numpy as np

import concourse.bass as bass
import concourse.mybir as mybir
import concourse.tile as tile
from concourse import bacc
from concourse.bass_utils import run_bass_kernel_spmd

F32 = mybir.dt.float32
F32R = mybir.dt.float32r
BF16 = mybir.dt.bfloat16
AF = mybir.ActivationFunctionType
ALU = mybir.AluOpType

B = 128      # batch (full, on every core)
E = 256      # embedding dim
H = 1024     # hidden
V = 128      # vocab
T = 512      # timesteps
HSL = 128    # hidden slice per core
G = 4 * HSL  # gate cols per core = 512
NCORES = 8

EXCH = "bf16"   # "f32r" | "bf16": dtype of weights + h-exchange payload

# gate order within each core's G columns: (i, o, f, j)
# reference order in W is (i, j, f, o) -> permutation of source blocks:
GATE_PERM = [0, 3, 2, 1]
CI, CO, CF, CJ = 0, 128, 256, 384  # column offsets of i/o/f/j blocks


def _np_dt(exch):
    if exch == "bf16":
        return mybir.dt.np(BF16)
    return np.float32


def _host_prep(inputs, exch=None):
    """Slice/transform full inputs into 8 per-core input maps."""
    exch = EXCH if exch is None else exch
    wdt = _np_dt(exch)
    ib = np.asarray(inputs["input_batch"])            # [B, T] int32
    lens = np.asarray(inputs["input_lengths"])        # [B]
    emb = np.asarray(inputs["char_embeddings"], dtype=np.float32)  # [V, E]
    W0 = np.asarray(inputs["W0"], dtype=np.float32)   # [E+H, 4H]
    b0 = np.asarray(inputs["b0"], dtype=np.float32)
    W1 = np.asarray(inputs["W1"], dtype=np.float32)   # [2H, 4H]
    b1 = np.asarray(inputs["b1"], dtype=np.float32)

    def gate_cols(W, k):
        return np.concatenate(
            [W[:, g * H + k * HSL: g * H + (k + 1) * HSL] for g in GATE_PERM],
            axis=1,
        )

    def gate_cols_b(b, k):
        return np.concatenate(
            [b[g * H + k * HSL: g * H + (k + 1) * HSL] for g in GATE_PERM]
        )

    tok_f32 = ib[:, :T].astype(np.float32)            # [B, T]
    iota_free = np.tile(np.arange(V, dtype=np.float32)[None, :], (B, 1))
    # capture mask: fires once, at each sequence's last valid step
    msel = (np.arange(T)[None, :] == (lens[:, None] - 1)).astype(np.float32)
    iota_p = np.arange(V, dtype=np.float32)[:, None]  # [V, 1]
    ident = np.eye(128, dtype=np.float32)

    in_maps = []
    for k in range(NCORES):
        W0c = gate_cols(W0, k)                        # [E+H, 512] (i,o,f,j)
        b0c = gate_cols_b(b0, k).copy()               # [512]
        b0c[CF:CF + HSL] += 1.0                       # forget bias layer 0
        ewb = emb @ W0c[:E] + b0c[None, :]            # [V, 512]
        w0h = W0c[E:]                                 # [1024, 512]
        w0h_t = np.concatenate(
            [w0h[j * 128: (j + 1) * 128] for j in range(8)], axis=1)
        W1c = gate_cols(W1, k)                        # [2048, 512]
        b1c = gate_cols_b(b1, k)
        # k-tile blocks: 0..7 = nh0 rows, 8..15 = h1 rows
        w1_t = np.concatenate(
            [W1c[j * 128: (j + 1) * 128] for j in range(16)], axis=1)
        b1_full = np.tile(b1c[None, :], (128, 1)).astype(np.float32)
        in_maps.append({
            "ewb": ewb.astype(wdt),
            "w0h": w0h_t.astype(wdt),
            "w1": w1_t.astype(wdt),
            "b1full": b1_full,
            "tok": tok_f32,
            "iotaf": iota_free,
            "msel": msel,
            "iotap": iota_p,
            "ident": ident,
        })
    has_b1 = bool(np.any(b1 != 0.0))
    return in_maps, has_b1


def build_kernel(has_b1=False, ag_mode="dual", exch=None):
    """Build + compile the SPMD Bass kernel for all 8 cores.

    ag_mode: "dual" (two staggered AGs/step) or "fake" (no collectives;
    local copies — WRONG results, compute-floor measurement only).
    """
    exch = EXCH if exch is None else exch
    XDT = BF16 if exch == "bf16" else F32R
    nc = bacc.Bacc("TRN2", target_bir_lowering=False, debug=False,
                   num_devices=NCORES)

    # ---- I/O ----
    d_ewb = nc.dram_tensor("ewb", [V, G], XDT, kind="ExternalInput")
    d_w0h = nc.dram_tensor("w0h", [128, 8 * G], XDT, kind="ExternalInput")
    d_w1 = nc.dram_tensor("w1", [128, 16 * G], XDT, kind="ExternalInput")
    d_b1 = nc.dram_tensor("b1full", [128, G], F32, kind="ExternalInput")
    d_tok = nc.dram_tensor("tok", [B, T], F32, kind="ExternalInput")
    d_iota = nc.dram_tensor("iotaf", [B, V], F32, kind="ExternalInput")
    d_msel = nc.dram_tensor("msel", [B, T], F32, kind="ExternalInput")
    d_iotap = nc.dram_tensor("iotap", [V, 1], F32, kind="ExternalInput")
    d_ident = nc.dram_tensor("ident", [128, 128], F32, kind="ExternalInput")
    d_out = nc.dram_tensor("out", [B, HSL], F32, kind="ExternalOutput")

    # ---- persistent SBUF ----
    sb_ewb = nc.alloc_sbuf_tensor("sb_ewb", [V, G], XDT)
    sb_w0h = nc.alloc_sbuf_tensor("sb_w0h", [128, 8 * G], XDT)
    sb_w1 = nc.alloc_sbuf_tensor("sb_w1", [128, 16 * G], XDT)
    sb_b1 = nc.alloc_sbuf_tensor("sb_b1", [128, G], F32)
    sb_tok = nc.alloc_sbuf_tensor("sb_tok", [B, T], F32)
    sb_iota = nc.alloc_sbuf_tensor("sb_iota", [B, V], F32)
    sb_msel = nc.alloc_sbuf_tensor("sb_msel", [B, T], F32)
    sb_iotap = nc.alloc_sbuf_tensor("sb_iotap", [V, 1], F32)
    sb_ident = nc.alloc_sbuf_tensor("sb_ident", [128, 128], F32)
    c0 = nc.alloc_sbuf_tensor("c0", [B, HSL], F32)
    c1 = nc.alloc_sbuf_tensor("c1", [B, HSL], F32)
    # hpair: [h0 | h1] staging in exchange dtype, written by the cells
    hpair = nc.alloc_sbuf_tensor("hpair", [B, 2 * HSL], XDT)
    h1f = nc.alloc_sbuf_tensor("h1f", [B, HSL], F32)
    outacc = nc.alloc_sbuf_tensor("outacc", [B, HSL], F32)

    with tile.TileContext(nc) as tc, ExitStack() as ctx:
        # ---- load weights/constants ----
        for sb, d in [(sb_ewb, d_ewb), (sb_w0h, d_w0h), (sb_w1, d_w1),
                      (sb_b1, d_b1), (sb_tok, d_tok), (sb_iota, d_iota),
                      (sb_msel, d_msel), (sb_iotap, d_iotap),
                      (sb_ident, d_ident)]:
            nc.sync.dma_start(sb[:], d[:])
        for st in (c0, c1, h1f, outacc):
            nc.vector.memset(st[:], 0.0)
        nc.vector.tensor_copy(hpair[:, 0:HSL], c0[:])
        nc.vector.tensor_copy(hpair[:, HSL:2 * HSL], c1[:])

        # ---- pools ----
        ps_z0 = ctx.enter_context(tc.tile_pool(name="psz0", bufs=2, space="PSUM"))
        ps_z1 = ctx.enter_context(tc.tile_pool(name="psz1", bufs=2, space="PSUM"))
        ps_tp = ctx.enter_context(tc.tile_pool(name="pstp", bufs=1, space="PSUM"))
        pool = ctx.enter_context(tc.tile_pool(name="work", bufs=3))
        rp0 = ctx.enter_context(tc.tile_pool(name="recv0", bufs=2))
        dram = ctx.enter_context(tc.tile_pool(name="dram", bufs=2, space="DRAM"))

        def do_ag(cin, cout):
            if ag_mode == "local":
                # timing-only: no exchange; block 0 fed from local cin
                nc.gpsimd.dma_start(cout[0:B, :], cin[:])
            elif ag_mode == "fake":
                for j in range(NCORES):
                    nc.gpsimd.dma_start(
                        cout[j * B:(j + 1) * B, :], cin[:])
            else:
                nc.gpsimd.collective_compute(
                    "AllGather", ALU.bypass,
                    replica_groups=[list(range(NCORES))],
                    ins=[cin[:].opt()], outs=[cout[:].opt()],
                )

        def cell(z, cst, hst, layer):
            """Pure LSTM cell on PSUM gates z [B, G] in (i,o,f,j) order;
            updates cst/hst in place. No length masking: states evolve
            freely past each sequence's end; the output is captured at
            its firing step instead (outacc += msel_t * h1)."""
            sig = pool.tile([B, 3 * HSL], F32, tag=f"sig{layer}")
            if layer == 0:
                # f-bias folded into EWb: one fused sigmoid over i|o|f
                nc.scalar.activation(sig[:], z[:, 0:CJ], AF.Sigmoid)
            else:
                nc.scalar.activation(sig[:, 0:CF], z[:, 0:CF], AF.Sigmoid)
                nc.scalar.activation(sig[:, CF:CJ], z[:, CF:CJ], AF.Sigmoid,
                                     bias=1.0)
            tanj = pool.tile([B, HSL], F32, tag=f"tanj{layer}")
            nc.scalar.activation(tanj[:], z[:, CJ:CJ + HSL], AF.Tanh)
            # c = c*sigf + sigi*tanj
            u = pool.tile([B, HSL], F32, tag=f"u{layer}")
            nc.vector.tensor_mul(u[:], sig[:, 0:CO], tanj[:])
            cm = pool.tile([B, HSL], F32, tag=f"cm{layer}")
            nc.vector.tensor_mul(cm[:], cst, sig[:, CF:CJ])
            nc.vector.tensor_add(cst, cm[:], u[:])
            # h = tanh(c) * sigo  (hst may be a bf16 slice of hpair)
            tanc = pool.tile([B, HSL], F32, tag=f"tanc{layer}")
            nc.scalar.activation(tanc[:], cst, AF.Tanh)
            nc.vector.tensor_mul(hst, tanc[:], sig[:, CO:CF])

        # merged exchange: one AG/step carries [h0(t) | h1(t-1)] in
        # [B, 2*HSL] layout (no sender-side transpose); the readback DMA
        # transposes via the X-bar into rh0/rh1 [h-dim, (j b)].
        def rblk0(j):
            return rh0[:, j * B:(j + 1) * B]

        def rblk1(j):
            return rh1[:, j * B:(j + 1) * B]

        rh0 = rh1 = None

        for t in range(T + 1):
            # ---- one-hot for x_t, direct in [V, B] layout ----
            if t < T:
                ohbt = pool.tile([B, V], F32, tag="ohbt")
                nc.vector.tensor_scalar(
                    ohbt[:], sb_iota[:], sb_tok[:, t:t + 1], None,
                    ALU.is_equal)
                poh = ps_tp.tile([V, B], F32, tag="poh")
                nc.tensor.transpose(poh[:], ohbt[:], sb_ident[:])
                ohT = pool.tile([V, B], XDT, tag="ohT")
                nc.scalar.copy(ohT[:], poh[:])

                # ---- z0(t) = oh @ EWb + h0(t-1) @ W0h ----
                z0 = ps_z0.tile([B, G], F32, tag="z0")
                nc.tensor.matmul(z0[:], ohT[:], sb_ewb[:],
                                 start=True, stop=(t == 0))
                if t > 0:
                    for j in range(8):
                        nc.tensor.matmul(
                            z0[:], rblk0(j),
                            sb_w0h[:, j * G:(j + 1) * G],
                            start=False, stop=(j == 7))

            # ---- z1(t-1) = nh0(t-1) @ W1a + h1(t-2) @ W1b ----
            if t >= 1:
                z1 = ps_z1.tile([B, G], F32, tag="z1")
                for j in range(8):
                    nc.tensor.matmul(
                        z1[:], rblk0(j),
                        sb_w1[:, j * G:(j + 1) * G],
                        start=(j == 0), stop=(t == 1 and j == 7))
                if t >= 2:
                    for j in range(8):
                        nc.tensor.matmul(
                            z1[:], rblk1(j),
                            sb_w1[:, (8 + j) * G:(9 + j) * G],
                            start=False, stop=(j == 7),
                            skip_group_check=True)

            # ---- cell0(t) -> writes hpair[:, 0:HSL] (bf16/exch dtype) ----
            if t < T:
                cell(z0, c0[:], hpair[:, 0:HSL], 0)

            # ---- cell1(t-1) -> h1f (f32), capture, cast into hpair ----
            if t >= 1:
                if has_b1:
                    zb = pool.tile([B, G], F32, tag="zb")
                    nc.vector.tensor_add(zb[:], z1[:], sb_b1[:])
                    z1ap = zb
                else:
                    z1ap = z1
                cell(z1ap, c1[:], h1f[:], 1)
                # capture h1(t-1) into the output at its firing step
                nc.vector.scalar_tensor_tensor(
                    outacc[:], h1f[:], sb_msel[:, t - 1:t], outacc[:],
                    ALU.mult, ALU.add)
                if t < T:
                    nc.vector.tensor_copy(hpair[:, HSL:2 * HSL], h1f[:])

            # ---- single AG + transposing readback ----
            if t < T:
                cin = dram.tile([B, 2 * HSL], XDT, tag="cin")
                nc.sync.dma_start(cin[:], hpair[:])
                cout = dram.tile([NCORES * B, 2 * HSL], XDT, tag="cout")
                do_ag(cin, cout)
                rh0 = rp0.tile([128, NCORES * B], XDT, tag="rh0")
                nc.sync.dma_start_transpose(rh0[:], cout[:, 0:HSL])
                rh1 = rp0.tile([128, NCORES * B], XDT, tag="rh1")
                nc.scalar.dma_start_transpose(rh1[:], cout[:, HSL:2 * HSL])

        # ---- output ----
        nc.sync.dma_start(d_out[:], outacc[:])

    nc.compile()
    return nc


_CACHE = {}


def kernel(**inputs) -> np.ndarray:
    """Full-input entry point: returns [B, H] fp32 encoder output."""
    in_maps, has_b1 = _host_prep(inputs)
    key = ("nc", has_b1, EXCH)
    if key not in _CACHE:
        _CACHE[key] = build_kernel(has_b1=has_b1)
    nc = _CACHE[key]
    res = run_bass_kernel_spmd(nc, in_maps, core_ids=list(range(NCORES)))
    out = np.concatenate(
        [res.results[k]["out"] for k in range(NCORES)], axis=1)
    return out.astype(np.float32)



# revision 38
# speedup vs baseline: 1.0405x; 1.0405x over previous
"""nn_EncoderModel: 2-layer LSTM encoder (B=128, T=512, E=256, H=1024)
on 8 trn2 NeuronCores — v2.5.

Hidden-dim model parallelism (core k owns h-dims [128k,128(k+1)) of
both layers), layer 1 lagged one step behind layer 0, ONE merged
AllGather per step carrying [h0(t); h1(t-1)] in bf16 (the AG here is
latency+size bound: 32KB ~6us vs 128KB ~14us serial). No per-step
length masking: states evolve freely past each sequence's end and the
output is captured at its firing step (outacc += msel_t * h1), which
is mathematically identical to dynamic_rnn's freeze-and-read-last. Matmuls run in float32r (1 cycle/row; fp32
is 4) or bf16. Cell elementwise math is fused into
scalar_tensor_tensor/tensor_scalar ops with the dynamic_rnn length mask
applied as per-partition scalars. Embedding lookup folds into the
layer-0 matmul via the one-hot trick; the one-hot is built directly in
[V, B] layout from a pre-transposed token table (no PE transpose).

Queues: PE = matmuls + state transposes; ACT = activations (+ ohT
fallback copy); DVE = cell elementwise + stage copies; gpsimd = cin
DMAs + collectives; SP = readbacks only. This keeps every compute
queue free of waits on collective completion.
"""

from contextlib import ExitStack

import numpy as np

import concourse.bass as bass
import concourse.mybir as mybir
import concourse.tile as tile
from concourse import bacc
from concourse.bass_utils import run_bass_kernel_spmd

F32 = mybir.dt.float32
F32R = mybir.dt.float32r
BF16 = mybir.dt.bfloat16
AF = mybir.ActivationFunctionType
ALU = mybir.AluOpType

B = 128      # batch (full, on every core)
E = 256      # embedding dim
H = 1024     # hidden
V = 128      # vocab
T = 512      # timesteps
HSL = 128    # hidden slice per core
G = 4 * HSL  # gate cols per core = 512
NCORES = 8

EXCH = "bf16"   # "f32r" | "bf16": dtype of weights + h-exchange payload

# gate order within each core's G columns: (i, o, f, j)
# reference order in W is (i, j, f, o) -> permutation of source blocks:
GATE_PERM = [0, 3, 2, 1]
CI, CO, CF, CJ = 0, 128, 256, 384  # column offsets of i/o/f/j blocks


def _np_dt(exch):
    if exch == "bf16":
        return mybir.dt.np(BF16)
    return np.float32


def _host_prep(inputs, exch=None):
    """Slice/transform full inputs into 8 per-core input maps."""
    exch = EXCH if exch is None else exch
    wdt = _np_dt(exch)
    ib = np.asarray(inputs["input_batch"])            # [B, T] int32
    lens = np.asarray(inputs["input_lengths"])        # [B]
    emb = np.asarray(inputs["char_embeddings"], dtype=np.float32)  # [V, E]
    W0 = np.asarray(inputs["W0"], dtype=np.float32)   # [E+H, 4H]
    b0 = np.asarray(inputs["b0"], dtype=np.float32)
    W1 = np.asarray(inputs["W1"], dtype=np.float32)   # [2H, 4H]
    b1 = np.asarray(inputs["b1"], dtype=np.float32)

    def gate_cols(W, k):
        return np.concatenate(
            [W[:, g * H + k * HSL: g * H + (k + 1) * HSL] for g in GATE_PERM],
            axis=1,
        )

    def gate_cols_b(b, k):
        return np.concatenate(
            [b[g * H + k * HSL: g * H + (k + 1) * HSL] for g in GATE_PERM]
        )

    tok_f32 = ib[:, :T].astype(np.float32)            # [B, T]
    iota_free = np.tile(np.arange(V, dtype=np.float32)[None, :], (B, 1))
    # capture mask: fires once, at each sequence's last valid step
    msel = (np.arange(T)[None, :] == (lens[:, None] - 1)).astype(np.float32)
    iota_p = np.arange(V, dtype=np.float32)[:, None]  # [V, 1]
    ident = np.eye(128, dtype=np.float32)

    in_maps = []
    for k in range(NCORES):
        W0c = gate_cols(W0, k)                        # [E+H, 512] (i,o,f,j)
        b0c = gate_cols_b(b0, k).copy()               # [512]
        b0c[CF:CF + HSL] += 1.0                       # forget bias layer 0
        ewb = emb @ W0c[:E] + b0c[None, :]            # [V, 512]
        w0h = W0c[E:]                                 # [1024, 512]
        w0h_t = np.concatenate(
            [w0h[j * 128: (j + 1) * 128] for j in range(8)], axis=1)
        W1c = gate_cols(W1, k)                        # [2048, 512]
        b1c = gate_cols_b(b1, k)
        # k-tile blocks: 0..7 = nh0 rows, 8..15 = h1 rows
        w1_t = np.concatenate(
            [W1c[j * 128: (j + 1) * 128] for j in range(16)], axis=1)
        b1_full = np.tile(b1c[None, :], (128, 1)).astype(np.float32)
        # precomputed x-contribution: z0x[t, b, :] = ewb[tok[b, t], :]
        z0x = np.ascontiguousarray(
            ewb.astype(wdt)[ib.T]).reshape(T * B, ewb.shape[1])
        in_maps.append({
            "z0x": z0x,
            "w0h": w0h_t.astype(wdt),
            "w1": w1_t.astype(wdt),
            "b1full": b1_full,
            "msel": msel,
            "ident": ident,
            "identb": ident.astype(wdt),
        })
    has_b1 = bool(np.any(b1 != 0.0))
    return in_maps, has_b1


def build_kernel(has_b1=False, ag_mode="dual", exch=None):
    """Build + compile the SPMD Bass kernel for all 8 cores.

    ag_mode: "dual" (two staggered AGs/step) or "fake" (no collectives;
    local copies — WRONG results, compute-floor measurement only).
    """
    exch = EXCH if exch is None else exch
    XDT = BF16 if exch == "bf16" else F32R
    nc = bacc.Bacc("TRN2", target_bir_lowering=False, debug=False,
                   num_devices=NCORES)

    # ---- I/O ----
    d_z0x = nc.dram_tensor("z0x", [T * B, G], XDT, kind="ExternalInput")
    d_w0h = nc.dram_tensor("w0h", [128, 8 * G], XDT, kind="ExternalInput")
    d_w1 = nc.dram_tensor("w1", [128, 16 * G], XDT, kind="ExternalInput")
    d_b1 = nc.dram_tensor("b1full", [128, G], F32, kind="ExternalInput")
    d_msel = nc.dram_tensor("msel", [B, T], F32, kind="ExternalInput")
    d_ident = nc.dram_tensor("ident", [128, 128], F32, kind="ExternalInput")
    d_identb = nc.dram_tensor("identb", [128, 128], XDT,
                              kind="ExternalInput")
    d_out = nc.dram_tensor("out", [B, HSL], F32, kind="ExternalOutput")

    # ---- persistent SBUF ----
    sb_w0h = nc.alloc_sbuf_tensor("sb_w0h", [128, 8 * G], XDT)
    sb_w1 = nc.alloc_sbuf_tensor("sb_w1", [128, 16 * G], XDT)
    sb_b1 = nc.alloc_sbuf_tensor("sb_b1", [128, G], F32)
    sb_msel = nc.alloc_sbuf_tensor("sb_msel", [B, T], F32)
    sb_ident = nc.alloc_sbuf_tensor("sb_ident", [128, 128], F32)
    sb_identb = nc.alloc_sbuf_tensor("sb_identb", [128, 128], XDT)
    c0 = nc.alloc_sbuf_tensor("c0", [B, HSL], F32)
    c1 = nc.alloc_sbuf_tensor("c1", [B, HSL], F32)
    h0bt = nc.alloc_sbuf_tensor("h0bt", [B, HSL], F32)
    h1bt = nc.alloc_sbuf_tensor("h1bt", [B, HSL], F32)
    zrow = nc.alloc_sbuf_tensor("zrow", [HSL, B], F32)
    outacc = nc.alloc_sbuf_tensor("outacc", [B, HSL], F32)

    with tile.TileContext(nc) as tc, ExitStack() as ctx:
        # ---- load weights/constants ----
        for sb, d in [(sb_w0h, d_w0h), (sb_w1, d_w1),
                      (sb_b1, d_b1), (sb_msel, d_msel),
                      (sb_ident, d_ident), (sb_identb, d_identb)]:
            nc.sync.dma_start(sb[:], d[:])
        for st in (c0, c1, h0bt, h1bt, zrow, outacc):
            nc.vector.memset(st[:], 0.0)

        # ---- pools ----
        ps_z0 = ctx.enter_context(tc.tile_pool(name="psz0", bufs=3, space="PSUM"))
        ps_z1 = ctx.enter_context(tc.tile_pool(name="psz1", bufs=3, space="PSUM"))
        ps_tp = ctx.enter_context(tc.tile_pool(name="pstp", bufs=1, space="PSUM"))
        pool = ctx.enter_context(tc.tile_pool(name="work", bufs=4))
        zxp = ctx.enter_context(tc.tile_pool(name="zx", bufs=4))
        rp0 = ctx.enter_context(tc.tile_pool(name="recv0", bufs=3))
        dram = ctx.enter_context(tc.tile_pool(name="dram", bufs=3, space="DRAM"))

        def do_ag(cin, cout):
            if ag_mode == "local":
                # timing-only: no exchange; block 0 fed from local cin
                nc.gpsimd.dma_start(cout[0:2 * HSL, :], cin[:])
            elif ag_mode == "fake":
                for j in range(NCORES):
                    nc.gpsimd.dma_start(
                        cout[j * 2 * HSL:(j + 1) * 2 * HSL, :], cin[:])
            else:
                nc.gpsimd.collective_compute(
                    "AllGather", ALU.bypass,
                    replica_groups=[list(range(NCORES))],
                    ins=[cin[:].opt()], outs=[cout[:].opt()],
                )

        def cell(z, cst, hst, layer):
            """Pure LSTM cell on PSUM gates z [B, G] in (i,o,f,j) order;
            updates cst/hst in place. No length masking: states evolve
            freely past each sequence's end; the output is captured at
            its firing step instead (outacc += msel_t * h1)."""
            sig = pool.tile([B, 3 * HSL], F32, tag=f"sig{layer}")
            if layer == 0:
                # f-bias folded into EWb: one fused sigmoid over i|o|f
                nc.scalar.activation(sig[:], z[:, 0:CJ], AF.Sigmoid)
            else:
                nc.scalar.activation(sig[:, 0:CF], z[:, 0:CF], AF.Sigmoid)
                nc.scalar.activation(sig[:, CF:CJ], z[:, CF:CJ], AF.Sigmoid,
                                     bias=1.0)
            tanj = pool.tile([B, HSL], F32, tag=f"tanj{layer}")
            nc.scalar.activation(tanj[:], z[:, CJ:CJ + HSL], AF.Tanh)
            # c = c*sigf + sigi*tanj
            u = pool.tile([B, HSL], F32, tag=f"u{layer}")
            nc.vector.tensor_mul(u[:], sig[:, 0:CO], tanj[:])
            cm = pool.tile([B, HSL], F32, tag=f"cm{layer}")
            nc.vector.tensor_mul(cm[:], cst, sig[:, CF:CJ])
            nc.vector.tensor_add(cst, cm[:], u[:])
            # h = tanh(c) * sigo
            tanc = pool.tile([B, HSL], F32, tag=f"tanc{layer}")
            nc.scalar.activation(tanc[:], cst, AF.Tanh)
            nc.vector.tensor_mul(hst, tanc[:], sig[:, CO:CF])

        # merged exchange: one AG/step carries [h0(t); h1(t-1)].
        # rh block layout: col (2j)*B..: core j's h0 slice; (2j+1)*B..: h1.
        def rblk0(j):
            return rh[:, (2 * j) * B:(2 * j) * B + B]

        def rblk1(j):
            return rh[:, (2 * j + 1) * B:(2 * j + 1) * B + B]

        rh = None

        for t in range(T + 1):
            # ---- one-hot for x_t, direct in [V, B] layout ----
            if t < T:
                ohbt = pool.tile([B, V], F32, tag="ohbt")
                nc.vector.tensor_scalar(
                    ohbt[:], sb_iota[:], sb_tok[:, t:t + 1], None,
                    ALU.is_equal)
                poh = ps_tp.tile([V, B], F32, tag="poh")
                nc.tensor.transpose(poh[:], ohbt[:], sb_ident[:])
                ohT = pool.tile([V, B], XDT, tag="ohT")
                nc.scalar.copy(ohT[:], poh[:])

                # ---- z0(t) = oh @ EWb + h0(t-1) @ W0h ----
                z0 = ps_z0.tile([B, G], F32, tag="z0")
                nc.tensor.matmul(z0[:], ohT[:], sb_ewb[:],
                                 start=True, stop=(t == 0))
                if t > 0:
                    for j in range(8):
                        nc.tensor.matmul(
                            z0[:], rblk0(j),
                            sb_w0h[:, j * G:(j + 1) * G],
                            start=False, stop=(j == 7))

            # ---- z1(t-1) = nh0(t-1) @ W1a + h1(t-2) @ W1b ----
            if t >= 1:
                z1 = ps_z1.tile([B, G], F32, tag="z1")
                for j in range(8):
                    nc.tensor.matmul(
                        z1[:], rblk0(j),
                        sb_w1[:, j * G:(j + 1) * G],
                        start=(j == 0), stop=(t == 1 and j == 7))
                if t >= 2:
                    for j in range(8):
                        nc.tensor.matmul(
                            z1[:], rblk1(j),
                            sb_w1[:, (8 + j) * G:(9 + j) * G],
                            start=False, stop=(j == 7),
                            skip_group_check=True)

            # ---- cell0(t) -> stage left half ----
            if t < T:
                stg = pool.tile([HSL, 2 * B], XDT, tag="stg")
                cell(z0, c0[:], h0bt[:], 0)
                tp0 = ps_tp.tile([HSL, B], F32, tag="tp0")
                nc.tensor.transpose(tp0[:], h0bt[:], sb_ident[:])
                nc.vector.tensor_copy(stg[:, 0:B], tp0[:])

            # ---- cell1(t-1) -> capture + stage right half ----
            if t >= 1:
                if has_b1:
                    zb = pool.tile([B, G], F32, tag="zb")
                    nc.vector.tensor_add(zb[:], z1[:], sb_b1[:])
                    z1ap = zb
                else:
                    z1ap = z1
                cell(z1ap, c1[:], h1bt[:], 1)
                # capture h1(t-1) into the output at its firing step
                nc.vector.scalar_tensor_tensor(
                    outacc[:], h1bt[:], sb_msel[:, t - 1:t], outacc[:],
                    ALU.mult, ALU.add)
                if t < T:
                    tp1 = ps_tp.tile([HSL, B], F32, tag="tp1")
                    nc.tensor.transpose(tp1[:], h1bt[:], sb_ident[:])
                    nc.vector.tensor_copy(stg[:, B:2 * B], tp1[:])
            elif t < T:
                nc.vector.tensor_copy(stg[:, B:2 * B], zrow[:])

            # ---- single AG + readback ----
            if t < T:
                cin = dram.tile([2 * HSL, B], XDT, tag="cin")
                nc.sync.dma_start(
                    cin[:].rearrange("(s p) b -> p s b", s=2),
                    stg[:].rearrange("p (s b) -> p s b", s=2))
                cout = dram.tile([NCORES * 2 * HSL, B], XDT, tag="cout")
                do_ag(cin, cout)
                rh = rp0.tile([128, NCORES * 2 * HSL], XDT, tag="rh")
                nc.sync.dma_start(
                    rh[:].rearrange("p (j s b) -> p j s b", j=NCORES, s=2),
                    cout[:].rearrange("(j s p) b -> p j s b", j=NCORES, s=2))

        # ---- output ----
        nc.sync.dma_start(d_out[:], outacc[:])

    nc.compile()
    return nc


_CACHE = {}


def kernel(**inputs) -> np.ndarray:
    """Full-input entry point: returns [B, H] fp32 encoder output."""
    in_maps, has_b1 = _host_prep(inputs)
    key = ("nc", has_b1, EXCH)
    if key not in _CACHE:
        _CACHE[key] = build_kernel(has_b1=has_b1)
    nc = _CACHE[key]
    res = run_bass_kernel_spmd(nc, in_maps, core_ids=list(range(NCORES)))
    out = np.concatenate(
        [res.results[k]["out"] for k in range(NCORES)], axis=1)
    return out.astype(np.float32)

